# revision 31
# baseline (speedup 1.0000x reference)
"""Bass/Tile TRN2 kernel for nn_Attention_48653389529729.

reference (jax):
    cat = concat([broadcast(hidden, (S,B,H)), encoder_output], axis=2)  # [S,B,2H]
    energy = tanh(einsum("sbi,hi->sbh", cat, W_attn) + b_attn)          # [S,B,H]
    scores = einsum("sbh,h->sb", energy, v)                             # [S,B]
    out = softmax(scores.T, axis=1)[:, None, :]                        # [B,1,S]

v2 design ([s,h] layout — no PE v-dot):
    The v1 kernel computed E in [h(part), s(free)] layout, which made the
    v-reduction a partition reduce that only the PE can do: 4 extra fp32r
    matmuls per batch = ~27us of PE time on top of the ~107us of main
    matmuls.  v2 flips the layout: E[s(part), h(free)] via
        matmul(lhsT=encT[i, s-chunk] (stationary), rhs=WeT[i, h] (moving))
    so the v-reduction becomes a FREE-axis reduce: one DVE
    tensor_tensor_reduce (th * v_bcast, accum=add) per [128, 500] chunk.
    PE now runs ONLY the 16 main matmuls per batch (ap=500 -> 208ns each).

    The bias a[b,h] = hidden[b] @ Wh.T + b_attn can't ride along in this
    layout (ACT bias is per-partition = per-s here), so it is folded into
    the encoder input ON THE HOST:  We @ (enc + delta[b]) = We@enc + a
    with delta[b] = pinv_hi(We) @ a[b] over the well-conditioned singular
    modes, and the rank-12 residual (worst 1/sigma modes) shipped through
    the 12 zero-pad contraction rows (i=500..511):  stationary rows get
    alpha[b,j] = u_j.a[b], moving rows get u_j.  Exact in real arithmetic;
    fp16 cost of the delta shift measured 8.0e-3 end-to-end (tol 2e-2).

    scores accumulate as sc_all[p, 32*sc + b] = scores[b, 128*sc + p];
    four per-sc PE transposes land each s-chunk on partitions 0..31 so
    plain DVE copies assemble V[b, s]; one [32, 512] softmax (no
    max-subtraction: |scores| <= ~50, far from exp overflow in f32) and a
    single output DMA finish the kernel.  v-reduce uses the custom-DVE
    ucode op affine_mul_reduce — the native TENSOR_TENSOR_REDUCE ISA
    opcode crashes the exec unit on this runtime.
"""

import sys

sys.path.insert(0, "/opt/trn_rl_repo")

import numpy as np

import concourse.mybir as mybir
import concourse.tile as tile
from concourse import bacc
from concourse.bass_utils import run_bass_kernel_spmd

F32 = mybir.dt.float32
F16 = mybir.dt.float16
TANH = mybir.ActivationFunctionType.Tanh
EXP = mybir.ActivationFunctionType.Exp
MULT = mybir.AluOpType.mult
ADD = mybir.AluOpType.add

S, B, H = 512, 256, 500
NCORES = 8
BL = B // NCORES  # 32 batches per core
KC = 128          # i (contraction) chunk size, zero-padded 500 -> 512
NKC = 4           # number of contraction chunks
NSC = 4           # number of s-partition chunks (512 / 128)
HP = NKC * KC     # padded i size (512)
RLOW = HP - H     # 12 low-sigma residual modes through the pad rows

_CACHE = {}


def _build(enc_bufs=5, gsz=2, th_bufs=6, scr_bufs=2, psumE_bufs=5, n_warm=8,
           variant="full"):
    nc = bacc.Bacc("TRN2", target_bir_lowering=False)

    # encT[p, k, b, sc, s] = enc'[i=128k+p, b, 128*sc+s]  (fp16, delta-shifted)
    encT_d = nc.dram_tensor("encT", [KC, NKC, BL, NSC, KC], F16,
                            kind="ExternalInput")
    # weT[p, k, h] = WeP[128k+p, h]  (fp16; rows 500.. are u_j residual rows)
    weT_d = nc.dram_tensor("weT", [KC, NKC, H], F16, kind="ExternalInput")
    # v broadcast across partitions: [128, 500] f32
    v_d = nc.dram_tensor("vb", [KC, H], F32, kind="ExternalInput")
    ident_d = nc.dram_tensor("ident", [KC, KC], F32, kind="ExternalInput")
    out_d = nc.dram_tensor("out", [BL, 1, S], F32, kind="ExternalOutput")

    # chunk schedule: first two batches load individually (k-split for the
    # very first) so the PE can start early; the rest in gsz-sized chunks
    sched = [(0, 1), (1, 1)] + [
        (b, min(gsz, BL - b)) for b in range(2, BL, gsz)
    ]
    b2c = {}
    for ci, (b0, n) in enumerate(sched):
        for o in range(n):
            b2c[b0 + o] = (ci, o)

    with tile.TileContext(nc) as tc:
        with (
            tc.tile_pool(name="singles", bufs=1) as singles,
            tc.tile_pool(name="encp1", bufs=2) as encp1,
            tc.tile_pool(name="encp", bufs=enc_bufs) as encp,
        ):
            # PE p-state warmup: memset on vector (keeps gpsimd free to
            # dispatch the bulk DMAs; sync/scalar HWDGE rings are ~40GB/s,
            # gpsimd SWDGE spreads descriptors across all DMA engines).
            warm = singles.tile([KC, S], F16, tag="warm")
            nc.vector.memset(warm, 0.0)

            def load_chunk(ci):
                b0, n = sched[ci]
                pool = encp1 if n == 1 else encp
                t = pool.tile([KC, NKC, n, NSC, KC], F16, tag=f"enc{n}")
                if ci == 0:
                    # sc=0 slice on the sync HWDGE ring: parallel with both
                    # scalar (weT0) and the gpsimd SWDGE queue, lands ~11us;
                    # sc=1..3 ride SWDGE, whose gen order shifts earlier
                    nc.sync.dma_start(
                        out=t[:, :, :, 0:1, :],
                        in_=encT_d[:, :, b0 : b0 + n, 0:1, :],
                    )
                    nc.gpsimd.dma_start(
                        out=t[:, :, :, 1:NSC, :],
                        in_=encT_d[:, :, b0 : b0 + n, 1:NSC, :],
                    )
                else:
                    nc.gpsimd.dma_start(
                        out=t, in_=encT_d[:, :, b0 : b0 + n, :, :]
                    )
                return t

            # per-k weT tiles so the first matmul only waits on k=0's
            # 128KB, not the whole 512KB; gen order weT0, chunk0-sc0,
            # weT1-3, then the rest
            weT = []
            for k in range(NKC):
                wt = singles.tile([KC, H], F16, tag=f"weT{k}")
                weT.append(wt)
            # scalar HWDGE ring (~40GB/s) runs in parallel with the
            # gpsimd SWDGE queue: 128KB lands ~11.1us
            nc.scalar.dma_start(out=weT[0], in_=weT_d[:, 0, :])
            chunks = {0: load_chunk(0)}
            for k in range(1, NKC):
                nc.gpsimd.dma_start(out=weT[k], in_=weT_d[:, k, :])
            for ci in (1, 2, 3):
                chunks[ci] = load_chunk(ci)

            v_bcast = singles.tile([KC, H], F32)
            nc.sync.dma_start(out=v_bcast, in_=v_d[:, :])
            ident = singles.tile([KC, KC], F32)
            nc.sync.dma_start(out=ident, in_=ident_d[:, :])
            # scores accumulator: sc_all[p, 32*sc + b] = scores[b, 128*sc+p]
            sc_all = singles.tile([KC, KC], F32)
            # preload the Exp activation table before the tail needs it
            exp_warm = singles.tile([1, 1], F32)
            nc.vector.memset(exp_warm, 0.0)
            nc.scalar.activation(out=exp_warm, in_=exp_warm, func=EXP, scale=1.0)

            with (
                tc.tile_pool(name="thp", bufs=th_bufs) as thp,
                tc.tile_pool(name="scrp", bufs=scr_bufs) as scrp,
                tc.tile_pool(name="sm", bufs=1) as sm,
                tc.tile_pool(name="ps_E", bufs=psumE_bufs, space="PSUM") as ps_E,
                tc.tile_pool(name="ps_T", bufs=3, space="PSUM") as ps_T,
            ):
                # warm matmuls borrow a ps_E-pool bank (PE-serial, no reader)
                psW = ps_E.tile([KC, S], F32, tag="psE")
                for _ in range(n_warm):
                    nc.tensor.matmul(
                        psW, warm[:, 0:KC], warm, start=True, stop=True
                    )

                NCHUNK = len(sched)
                for bi in range(BL):
                    ci, off = b2c[bi]
                    et = chunks[ci]
                    if off == 0:
                        for ca in (ci + 4, ci + 5):
                            if ca < NCHUNK and ca not in chunks and len(
                                chunks
                            ) < enc_bufs + 2:
                                chunks[ca] = load_chunk(ca)
                    for sc in range(NSC):
                        # full-bank (2048B) PSUM tile; matmul writes [:, :H]
                        psE = ps_E.tile([KC, S], F32, tag="psE")
                        for k in range(NKC):
                            nc.tensor.matmul(
                                psE[:, 0:H],
                                et[:, k, off, sc, :],
                                weT[k],
                                start=(k == 0),
                                stop=(k == NKC - 1),
                            )
                        if variant == "noact":
                            continue
                        th = thp.tile([KC, H], F32, tag="th")
                        nc.scalar.activation(out=th, in_=psE[:, 0:H], func=TANH)
                        if variant == "nodve":
                            continue
                        col = 32 * sc + bi
                        if variant == "nottr":
                            # bisect: plain unfused reduce instead of ttr
                            nc.vector.reduce_sum(
                                sc_all[:, col : col + 1],
                                th,
                                axis=mybir.AxisListType.X,
                            )
                        else:
                            # custom-DVE ucode op: out = (th*1+0)*v,
                            # accum_out = sum over free axis.  (The native
                            # TENSOR_TENSOR_REDUCE ISA opcode crashes the
                            # exec unit on this runtime.)
                            scr = scrp.tile([KC, H], F32, tag="scr")
                            nc.vector.affine_mul_reduce(
                                out=scr,
                                accum_out=sc_all[:, col : col + 1],
                                in0=th,
                                in1=v_bcast,
                                scale=1.0,
                                bias=0.0,
                            )
                    if off == sched[ci][1] - 1:
                        chunks.pop(ci, None)

                # ---- epilogue: 4 per-sc transposes land scores on
                # partitions 0..31, so plain DVE copies (not DMAs)
                # assemble V[b, s]; one softmax, one output DMA.
                # No max-subtraction: |scores| <= ~50 here, exp() is far
                # from f32 overflow, and softmax normalizes regardless.
                V = sm.tile([BL, S], F32, tag="V")
                if variant in ("noact", "nodve"):
                    # bisect: no scores were produced; fabricate some
                    nc.vector.memset(V, 0.125)
                else:
                    for sc in range(NSC):
                        psT = ps_T.tile([32, S], F32, tag="psT")
                        nc.tensor.transpose(
                            psT[:, 0:KC],
                            sc_all[:, 32 * sc : 32 * sc + 32],
                            ident,
                        )
                        nc.vector.tensor_copy(
                            V[:, KC * sc : KC * (sc + 1)], psT[:, 0:KC]
                        )
                probs = sm.tile([BL, S], F32, tag="probs")
                sums = sm.tile([BL, 1], F32, tag="sums")
                nc.scalar.activation(
                    out=probs,
                    in_=V,
                    func=EXP,
                    scale=1.0,
                    accum_out=sums,
                )
                rinv = sm.tile([BL, 1], F32, tag="rinv")
                nc.vector.reciprocal(rinv, sums)
                half = S // 2
                for hx in range(2):
                    sl = slice(hx * half, (hx + 1) * half)
                    nc.vector.tensor_scalar_mul(
                        probs[:, sl], probs[:, sl], rinv
                    )
                    nc.sync.dma_start(
                        out=out_d[:, 0:1, sl],
                        in_=probs[:, sl].rearrange(
                            "b (one s) -> b one s", one=1
                        ),
                    )

    nc.compile()
    return nc


def _get_nc(**kw):
    key = tuple(sorted(kw.items()))
    if key not in _CACHE:
        _CACHE[key] = _build(**kw)
    return _CACHE[key]


def kernel(hidden, encoder_output, W_attn, b_attn, v, **run_kw):
    hidden = np.asarray(hidden, dtype=np.float64)
    encoder_output = np.asarray(encoder_output, dtype=np.float32)
    W_attn = np.asarray(W_attn, dtype=np.float64)
    b_attn = np.asarray(b_attn, dtype=np.float64)
    v = np.asarray(v, dtype=np.float32)

    # ---- host-side bias fold (one-shot, f64) ----
    Wh, We = W_attn[:, :H], W_attn[:, H:]
    a = hidden[0] @ Wh.T + b_attn                       # [B, H]
    U, sig, Vt = np.linalg.svd(We)
    hi = slice(0, H - RLOW)
    # delta[b] = V diag(1/sig_hi) U_hi^T a[b]   -> We @ delta = P_hi a
    delta = Vt[hi].T @ ((U[:, hi].T @ a.T) / sig[hi, None])   # [H, B]
    alpha = U[:, H - RLOW :].T @ a.T                          # [RLOW, B]

    # enc'[i, b, s]: rows 0..499 = enc + delta (bcast over s), rows 500.. = alpha
    encp = np.empty((HP, B, S), dtype=np.float32)
    encp[:H] = encoder_output.transpose(2, 1, 0) + delta[:, :, None].astype(
        np.float32
    )
    encp[H:] = alpha[:, :, None].astype(np.float32)
    # encT[p, k, b, sc, s] = enc'[128k+p, b, 128*sc+s]  fp16
    encT = np.ascontiguousarray(
        encp.reshape(NKC, KC, B, NSC, KC).transpose(1, 0, 2, 3, 4)
    ).astype(np.float16)

    weP = np.empty((HP, H), dtype=np.float64)
    weP[:H] = We.T
    weP[H:] = U[:, H - RLOW :].T
    weT = np.ascontiguousarray(
        weP.reshape(NKC, KC, H).transpose(1, 0, 2)
    ).astype(np.float16)

    v_bcast = np.ascontiguousarray(
        np.broadcast_to(v[None, :], (KC, H))
    ).astype(np.float32)
    ident = np.eye(KC, dtype=np.float32)

    nc = _get_nc()
    in_maps = []
    for c in range(NCORES):
        sl = slice(c * BL, (c + 1) * BL)
        in_maps.append(
            {
                "encT": np.ascontiguousarray(encT[:, :, sl, :, :]),
                "weT": weT,
                "vb": v_bcast,
                "ident": ident,
            }
        )
    res = run_bass_kernel_spmd(
        nc, in_maps, core_ids=list(range(NCORES)), **run_kw
    )
    out = np.concatenate([res.results[c]["out"] for c in range(NCORES)], axis=0)
    if run_kw:
        return out.astype(np.float32), res
    return out.astype(np.float32)
